# revision 3
# baseline (speedup 1.0000x reference)
"""DynamicLinear (MoE routing) Trainium2 Bass kernel.

Math (per sample b):
    out[b] = sum_k attn[b,k] * (x[b] @ W[k].T + bias[k])
           = sum_k attn[b,k] * (x[b] @ W[k].T) + attn[b] @ bias

Sharding: 8 cores in a 2x4 grid over (batch, out_features).
Each core computes out[b_half, o_quarter] from x[b_half] (8 MiB bf16)
and W[:, o_quarter, :] (8 MiB bf16) -- no cross-core communication.

The host ships x and W pre-tiled and pre-cast to bf16 in the exact
SBUF layouts the kernel consumes (contraction dim on partitions,
contiguous per partition), so the device needs no casts and no
transposes. Matmuls run bf16 x bf16 with fp32 PSUM accumulation (the
compute roofline: 218.6 us/core; fp8 DoubleRow is only ~1.44x and
needs >=3 matmuls to stay under the 2e-2 error budget, a net loss).

v2 schedule changes vs the 248.5 us baseline, targeting the ~32 us of
non-roofline time (head 14.2, cold-HAM 2.3, mid gaps 1.8, tail 5.8):
- 16 zero-input warmup matmuls issued before any DMA-dependent work
  warm the PE HAM clock gate (1.2 -> 2.4 GHz) during the initial DMA
  fill, so real matmuls start warm.
- Expert-0 weights stream in 8 x 2-ii granules (256 KiB) consumed in
  arrival order, so the first real matmul needs only 0.25 MiB of W +
  one x tile instead of 2.5 MiB.
- DMA triggers spread across queues: W0/W1 + out stores on sync,
  x stream + attn on scalar, bias + W2/W3 on gpsimd (trigger
  instructions cost ~0.7 us each on a sequencer).
- The last-processed tile (k=3, t=15) runs as 384/128-column half
  passes with separate PSUM tiles, accs, and store queues, so the
  final combine+store tail shrinks from ~2.9 us to ~1.7 us.
"""

import numpy as np

_B, _K, _IN, _OUT = 4096, 4, 2048, 2048
_GRID_B, _GRID_O = 2, 4
_BL = _B // _GRID_B      # 2048 batch rows per core
_OL = _OUT // _GRID_O    # 512 out cols per core
_NBT = _BL // 128        # 16 b tiles
_NIT = _IN // 128        # 16 contraction tiles

_CACHE = {}
LAST_RESULTS = None


def _build_program():
    import concourse.bass as bass
    import concourse.tile as tile
    from concourse import bacc, mybir

    f32 = mybir.dt.float32
    MULT = mybir.AluOpType.mult
    ADD = mybir.AluOpType.add

    nc = bacc.Bacc("TRN2", target_bir_lowering=False, debug=False)
    bf16 = mybir.dt.bfloat16
    xT = nc.dram_tensor("xT", [_NBT, 128, _NIT, 128], bf16,
                        kind="ExternalInput").ap()
    attn = nc.dram_tensor("attn", [_BL, _K], f32, kind="ExternalInput").ap()
    wT = nc.dram_tensor("wT", [_K, 128, _NIT, _OL], bf16,
                        kind="ExternalInput").ap()
    bias = nc.dram_tensor("bias", [_K, _OL], f32, kind="ExternalInput").ap()
    out = nc.dram_tensor("out", [_BL, _OL], f32, kind="ExternalOutput").ap()

    G0 = 2            # ii-tiles per W granule, expert 0 (fine: fast start)
    GH = 4            # ii-tiles per W granule, experts 1..3
    _SPLIT = 384      # column split of the final (k=3, t=15) pass

    with tile.TileContext(nc) as tc:
        with (
            tc.tile_pool(name="wt0", bufs=_NIT // G0) as wt0p,
            tc.tile_pool(name="wt", bufs=3 * (_NIT // GH)) as wtp,
            tc.tile_pool(name="xt", bufs=_NBT) as xtp,
            tc.tile_pool(name="singles", bufs=1) as singles,
            tc.tile_pool(name="acc", bufs=_NBT - 1) as accp,
            tc.tile_pool(name="acc15", bufs=1) as acc15p,
            tc.tile_pool(name="psum", bufs=6, space="PSUM") as psump,
            tc.tile_pool(name="psumh", bufs=1, space="PSUM") as psumhp,
        ):
            # --- PE warmup: ~3.4 us of zero matmuls to flip the HAM
            # clock gate to 8/8 while the first weights stream in.
            warm = singles.tile([128, 512], bf16, name="warm")
            nc.vector.memset(warm, 0.0)
            ps_warm = psump.tile([128, 256], f32, tag="ps", name="ps_warm")
            for i in range(16):
                nc.tensor.matmul(
                    ps_warm, lhsT=warm[:, 0:128], rhs=warm[:, 0:256],
                    start=(i == 0), stop=(i == 15),
                )

            # --- loads ---
            def load_w0(h):
                # expert-0 granule: wt0[h][i_in, j, o], j in [0, G0)
                t_ = wt0p.tile([128, G0, _OL], bf16, tag="wt0",
                               name=f"wt0_{h}")
                nc.sync.dma_start(out=t_, in_=wT[0, :, h * G0:(h + 1) * G0])
                return t_

            def load_w(k, h):
                t_ = wtp.tile([128, GH, _OL], bf16, tag="wt",
                              name=f"wt{k}_{h}")
                q = nc.sync if k == 1 else nc.gpsimd
                q.dma_start(out=t_, in_=wT[k, :, h * GH:(h + 1) * GH])
                return t_

            def load_x(t):
                # xt[t][i_in, ii, b] = x[t*128 + b, ii*128 + i_in]
                t_ = xtp.tile([128, _NIT, 128], bf16, tag="xt",
                              name=f"xt{t}")
                nc.scalar.dma_start(out=t_, in_=xT[t])
                return t_

            # expert-0 granules first (critical path), then x stream
            wt0 = {h: load_w0(h) for h in range(_NIT // G0)}
            xts = {0: load_x(0)}

            # attn for all b_tiles, b on partitions (scalar queue, after
            # xt0): attn_sb[p, t, k] = attn[t*128 + p, k]
            attn_sb = singles.tile([128, _NBT, _K], f32)
            attn_src = bass.AP(
                tensor=attn.tensor,
                offset=attn.offset,
                ap=[[_K, 128], [128 * _K, _NBT], [1, _K]],
            )
            nc.scalar.dma_start(out=attn_sb, in_=attn_src)
            for t in range(1, _NBT):
                xts[t] = load_x(t)

            # bias replicated across all 128 partitions (gpsimd SWDGE)
            bias_rep = singles.tile([128, _K, _OL], f32)
            nc.gpsimd.dma_start(
                out=bias_rep,
                in_=bass.AP(
                    tensor=bias.tensor,
                    offset=bias.offset,
                    ap=[[0, 128], bias.ap[0], bias.ap[1]],
                ),
            )

            wt = {}
            for h in range(_NIT // GH):
                wt[(1, h)] = load_w(1, h)
            for k in (2, 3):
                for h in range(_NIT // GH):
                    wt[(k, h)] = load_w(k, h)

            def w_slice(k, ii, c0=0, c1=_OL):
                if k == 0:
                    return wt0[ii // G0][:, ii % G0, c0:c1]
                return wt[(k, ii // GH)][:, ii % GH, c0:c1]

            acc = [None] * _NBT      # full tiles for t < 15
            acc15 = [None, None]     # [0:_SPLIT], [_SPLIT:_OL] for t = 15

            def combine(k, t, ps_ap, a_sc, c0, c1, which):
                # acc update for columns [c0:c1); which selects the acc
                if t < _NBT - 1:
                    at = acc[t]
                else:
                    at = acc15[which]
                if k == 0:
                    nc.vector.tensor_scalar(
                        out=at, in0=bias_rep[:, 0, c0:c1],
                        scalar1=a_sc[:, 0:1], scalar2=None, op0=MULT,
                    )
                    for kk in range(1, _K):
                        nc.vector.scalar_tensor_tensor(
                            out=at, in0=bias_rep[:, kk, c0:c1],
                            scalar=a_sc[:, kk:kk + 1], in1=at,
                            op0=MULT, op1=ADD,
                        )
                nc.vector.scalar_tensor_tensor(
                    out=at, in0=ps_ap, scalar=a_sc[:, k:k + 1],
                    in1=at, op0=MULT, op1=ADD,
                )

            for k in range(_K):
                for t in range(_NBT):
                    xt = xts[t]
                    a_sc = attn_sb[:, t, :]
                    last_tile = (t == _NBT - 1)
                    if k == 0:
                        if last_tile:
                            acc15[0] = acc15p.tile(
                                [128, _SPLIT], f32, tag="accA", name="accA")
                            acc15[1] = acc15p.tile(
                                [128, _OL - _SPLIT], f32, tag="accB",
                                name="accB")
                        else:
                            acc[t] = accp.tile([128, _OL], f32, tag="acc",
                                               name=f"acc{t}")
                    if k == _K - 1 and last_tile:
                        # final pass split into two half passes so the
                        # tail is one small combine + 64 KiB store
                        for which, (c0, c1) in enumerate(
                                [(0, _SPLIT), (_SPLIT, _OL)]):
                            ph = psumhp.tile([128, c1 - c0], f32,
                                             tag=f"psh{which}",
                                             name=f"psh{which}")
                            for ii in range(_NIT):
                                nc.tensor.matmul(
                                    ph,
                                    lhsT=xt[:, ii, :],
                                    rhs=w_slice(k, ii, c0, c1),
                                    start=(ii == 0), stop=(ii == _NIT - 1),
                                )
                            combine(k, t, ph, a_sc, c0, c1, which)
                            q = nc.sync if which == 0 else nc.gpsimd
                            q.dma_start(
                                out=out[t * 128:(t + 1) * 128, c0:c1],
                                in_=acc15[which],
                            )
                        continue
                    ps = psump.tile([128, _OL], f32, tag="ps",
                                    name=f"ps{k}_{t}")
                    for ii in range(_NIT):
                        nc.tensor.matmul(
                            ps,
                            lhsT=xt[:, ii, :],
                            rhs=w_slice(k, ii),
                            start=(ii == 0), stop=(ii == _NIT - 1),
                        )
                    if last_tile:
                        combine(k, t, ps[:, 0:_SPLIT], a_sc, 0, _SPLIT, 0)
                        combine(k, t, ps[:, _SPLIT:_OL], a_sc, _SPLIT,
                                _OL, 1)
                    else:
                        combine(k, t, ps, a_sc, 0, _OL, 0)
                        if k == _K - 1:
                            nc.sync.dma_start(
                                out=out[t * 128:(t + 1) * 128, :],
                                in_=acc[t],
                            )

    nc.compile()
    return nc


def _get_program():
    if "nc" not in _CACHE:
        _CACHE["nc"] = _build_program()
    return _CACHE["nc"]


def _ensure_axon_hooks_importable():
    """bass_utils' trace branch imports antenv.axon_hooks, which the
    trimmed agent image may lack; stub it (hook=None) so a stray
    BASS_TRACE=1 degrades to an untraced run instead of crashing."""
    import sys
    import types

    try:
        import antenv.axon_hooks  # noqa: F401
        return
    except ImportError:
        pass
    mod = types.ModuleType("antenv.axon_hooks")
    mod._hook = None
    mod.get_axon_ntff_profile_hook = lambda: mod._hook

    def _set(h):
        mod._hook = h

    mod.set_axon_ntff_profile_hook = _set
    sys.modules["antenv.axon_hooks"] = mod
    try:
        import antenv
        antenv.axon_hooks = mod
    except ImportError:
        pass


def kernel(**inputs):
    global LAST_RESULTS
    from concourse.bass_utils import run_bass_kernel_spmd

    _ensure_axon_hooks_importable()

    x = np.ascontiguousarray(inputs["x"], dtype=np.float32)
    attn = np.ascontiguousarray(inputs["softmax_attention"], dtype=np.float32)
    w = np.ascontiguousarray(inputs["weight"], dtype=np.float32)
    b = np.ascontiguousarray(inputs["bias"], dtype=np.float32)

    nc = _get_program()
    in_maps = []
    for c in range(8):
        gb, go = divmod(c, _GRID_O)
        x_sl = x[gb * _BL:(gb + 1) * _BL]
        w_sl = w[:, go * _OL:(go + 1) * _OL, :]
        # tile-contiguous device layouts (see _build_program):
        # xT[t, i_in, ii, b_in] = x[t*128 + b_in, ii*128 + i_in]
        # wT[k, i_in, ii, o]    = W[k, o, ii*128 + i_in]
        import ml_dtypes
        xT = np.ascontiguousarray(
            x_sl.T.reshape(_NIT, 128, _NBT, 128).transpose(2, 1, 0, 3)
        ).astype(ml_dtypes.bfloat16)
        wTa = np.ascontiguousarray(
            w_sl.transpose(0, 2, 1)
            .reshape(_K, _NIT, 128, _OL).transpose(0, 2, 1, 3)
        ).astype(ml_dtypes.bfloat16)
        in_maps.append({
            "xT": xT,
            "attn": np.ascontiguousarray(attn[gb * _BL:(gb + 1) * _BL]),
            "wT": wTa,
            "bias": np.ascontiguousarray(b[:, go * _OL:(go + 1) * _OL]),
        })

    res = run_bass_kernel_spmd(nc, in_maps, list(range(8)))
    LAST_RESULTS = res

    full = np.empty((_B, _OUT), dtype=np.float32)
    for c in range(8):
        gb, go = divmod(c, _GRID_O)
        full[gb * _BL:(gb + 1) * _BL, go * _OL:(go + 1) * _OL] = \
            res.results[c]["out"]
    return full
